# revision 45
# baseline (speedup 1.0000x reference)
"""SAM-style global attention (1,64,64,768), 12 heads, on 8 TRN2 NeuronCores.

Sharding: 24 units of (head, query-half-of-2048). Core c owns units
[3c, 3c+3) = 1.5 heads of queries spanning exactly 2 heads. Each core emits a
partial projected output outT (768, 4096); host sums the 8 partials, adds
proj_b (+ the projected v-bias, token-independent), transposes.

SPMD trick: even cores' units form the pattern [(j0,half0),(j0,half1),
(j1,half0)]; odd cores' form [(j0,half1),(j1,half0),(j1,half1)]. One graph
must serve both, so odd cores get their TOKEN ORDER half-swapped on the host
(xT columns, rel_h gather built with the swapped coords, output columns
un-swapped on host). In swapped space every core sees the canonical pattern
[(0,0),(0,1),(1,0)] with local head 0 = the fully-owned head.

Device math per local head j (k-major scores, q-major context):
  Q' (128, 4096): [scale*q^T + qb ; RelW^T] (j=0) / swapped halves (j=1)
  K' (128, 4096): [k^T ; onehot(kw)]  (j=0) / [onehot(kw) ; k^T]   (j=1)
  S^T[k,q] = K'.T @ Q'  (+ fp8e4 DoubleRow matmul onehot(kh)/16 x 16*RelH^T)
           = scale*q.k + rel_w[q,kw] + rel_h[q,kh]
  E^T = exp(S^T) on ScalarE straight out of PSUM (|S| < ~3: no max needed)
  ctx[q,d] (q-major): lhsT = E^T q-slices, rhs = V'[k, 0:64]=v, V'[k,64]=1
           -> col 64 = softmax denominator; normalize via per-partition
           reciprocal + tensor_scalar mult on DVE (v-bias folded into host
           proj_b: softmax weights sum to 1, so ctx_full = ctx/den + vb and
           vb @ projW^T is a token-independent constant).
  PE-transpose ctx (128q, [j0 d|j1 d]) -> (128 hd, 128 q) stacking both local
  heads on the contraction axis; proj: outT += projT2^T @ ctx^T.
"""

import numpy as np
import ml_dtypes

NH, HD, Hh, Ww, DIM = 12, 64, 64, 64, 768
HW = Hh * Ww  # 4096
SCALE = HD ** -0.5
NCORES = 8
BF16 = ml_dtypes.bfloat16
F8E4 = ml_dtypes.float8_e4m3

USE_FP8_REL = True

LAST_EXEC_NS = None
_PROGRAM = None


def _core_units(c):
    units = [(u // 2, u % 2) for u in range(3 * c, 3 * c + 3)]
    heads = sorted({h for h, _ in units})
    return units, heads


def _prep_core_inputs(c, x, qkv_w, qkv_b, proj_w, rel_pos_h, rel_pos_w):
    f32 = np.float32
    units, heads = _core_units(c)
    swapped = (c % 2 == 1)
    if swapped:
        # local head 0 must be the fully-owned head = heads[1]
        h0, h1 = heads[1], heads[0]
    else:
        h0, h1 = heads[0], heads[1]

    xflat = x.reshape(HW, DIM).astype(f32)
    if swapped:
        xflat = np.concatenate([xflat[2048:], xflat[:2048]], axis=0)
    xT = np.ascontiguousarray(xflat.T).astype(BF16)  # (768, 4096)

    def wslice(base, h):
        return qkv_w[base + h * 64: base + h * 64 + 64, :].astype(f32)

    def pack_chunks(wa, wb):  # (64,768) x2 -> (6, 128, 128) [chunk, ic, cols]
        wt = np.concatenate([wa.T, wb.T], axis=1)  # (768, 128)
        return np.ascontiguousarray(
            wt.reshape(6, 128, 128)).astype(BF16)

    wk = pack_chunks(wslice(768, h0), wslice(768, h1))
    wq = pack_chunks(SCALE * wslice(0, h0), SCALE * wslice(0, h1))
    wv = pack_chunks(wslice(1536, h0), wslice(1536, h1))
    # one partition-major tensor, grouped by kind so the K-weights third
    # can ride its own early DMA: wall[p, kind*768 + i*128 : +128];
    # cols 2304:2432 hold the identity used by the PE transposes.
    wall = np.zeros((128, 19 * 128), dtype=BF16)
    for i in range(6):
        for kind, w in enumerate((wk, wq, wv)):
            wall[:, kind * 768 + i * 128: kind * 768 + i * 128 + 128] = w[i]
    wall[:, 2304:2432] = np.eye(128, dtype=BF16)

    # k-bias dropped (softmax invariant to per-row constants); v-bias folded
    # into proj_b on the host. Only the q-bias rides to the device.
    qb2 = np.concatenate([
        qkv_b[h0 * 64: h0 * 64 + 64],
        qkv_b[h1 * 64: h1 * 64 + 64],
    ]).astype(f32).reshape(128, 1)
    ball = np.ascontiguousarray(SCALE * qb2)  # (128, 1) f32

    # rel gathers in (possibly swapped) coordinates. The h-coordinate of
    # token t_new is perm(t_new // 64) where perm(a) = (a+32)%64 for odd
    # cores; the w-coordinate is unchanged.  rel value needs ORIGINAL coords.
    a = np.arange(64)
    perm = ((a + 32) % 64) if swapped else a
    idx_h = perm[:, None] - perm[None, :] + 63     # (qh_new, kh_new)
    idx_w = a[:, None] - a[None, :] + 63           # (qw, kw)

    def gather(tab, idx, scale):
        g = np.transpose(tab[idx], (2, 0, 1)).reshape(HD, HW) * scale
        return np.ascontiguousarray(
            np.concatenate([g, g], axis=0)).astype(BF16)  # (128, 4096)

    # rel_h scaled by 8 (cancel the SCALE folded into Qp) * 16 (fp8 headroom
    # pairing with the 1/16 onehot); rel_w by 8 only (stays bf16-exact in the
    # main S contraction).
    relh16 = gather(rel_pos_h, idx_h, 128.0)
    relw = gather(rel_pos_w, idx_w, 8.0)
    relpack = np.ascontiguousarray(
        np.concatenate([relh16, relw], axis=1))  # (128, 2*HW) bf16

    k = np.arange(HW)
    ohkw = np.ascontiguousarray(
        (k[None, :] % 64 == a[:, None])).astype(BF16)   # (64, 4096)

    # fp8 kh-onehot for the DoubleRow rel_h matmul: ohkh8[p, i*HW + k] =
    # 1/16 iff k//64 == i*32 + p  (i = DoubleRow k-tile index)
    kh = k // 64
    oh = np.zeros((32, 2, HW), dtype=f32)
    oh[kh % 32, kh // 32, k] = 1.0 / 16.0
    ohkh8 = np.ascontiguousarray(oh.reshape(32, 2 * HW)).astype(F8E4)

    # proj weights with both local heads stacked on the contraction axis
    projT2 = np.ascontiguousarray(np.concatenate(
        [proj_w[:, h * 64: h * 64 + 64].T.astype(f32) for h in (h0, h1)],
        axis=0)).astype(BF16)  # (128, 768)

    return dict(xT=xT, wall=wall, ball=ball, relpack=relpack,
                ohkh8=ohkh8, ohkw=ohkw, projT2=projT2)


def _build_program():
    import concourse.bacc as bacc
    import concourse.tile as tile
    import concourse.mybir as mybir

    f32 = mybir.dt.float32
    bf16 = mybir.dt.bfloat16
    f8e4 = mybir.dt.float8e4
    AF = mybir.ActivationFunctionType
    PM = mybir.MatmulPerfMode

    nc = bacc.Bacc("TRN2", target_bir_lowering=False, debug=False,
                   enable_asserts=False, num_devices=NCORES)

    def din(name, shape, dt=bf16):
        return nc.dram_tensor(name, list(shape), dt, kind="ExternalInput").ap()

    xT_d = din("xT", (DIM, HW))
    wall_d = din("wall", (128, 19 * 128))
    ball_d = din("ball", (128, 1), f32)
    relpack_d = din("relpack", (128, 2 * HW))
    ohkh8_d = din("ohkh8", (32, 2 * HW), f8e4)
    ohkw_d = din("ohkw", (64, HW))
    projT2_d = din("projT2", (128, 768))
    outT_d = nc.dram_tensor("outT", [DIM, HW], bf16,
                            kind="ExternalOutput").ap()

    # canonical units in emission order: u0=(j0,h0), u1=(j1,h0), u2=(j0,h1)
    sched = [(0, 0), (1, 0), (0, 1)]
    RHW = [HW, 2048]

    with tile.TileContext(nc) as tc, \
         tc.tile_pool(name="persist", bufs=1) as P:
        # ---- DMAs, dependency-ordered so the first scores + exp can start
        # ~12us in instead of waiting for the full 6MB xT stream. relw/relh
        # have duplicated row halves: only the half each consumer needs is
        # fetched when it is needed.
        wall_s = P.tile([128, 19 * 128], bf16, name="wall")
        nc.sync.dma_start(wall_s[:, 0:768], wall_d[:, 0:768])
        wk_s = [wall_s[:, i * 128: i * 128 + 128] for i in range(6)]
        wq_s = [wall_s[:, 768 + i * 128: 768 + i * 128 + 128]
                for i in range(6)]
        wv_s = [wall_s[:, 1536 + i * 128: 1536 + i * 128 + 128]
                for i in range(6)]
        ident_s = wall_s[:, 2304:2432]

        # xT as ONE wide tile so a whole token-block (all 6 contraction
        # chunks) rides a single 3D-AP DMA — HWDGE issue (625ns/DMA) was
        # pacing the start more than the transfers themselves.
        xT_s = P.tile([128, 6 * HW], bf16, name="xT")
        xT = [xT_s[:, i * HW: i * HW + HW] for i in range(6)]
        xT_sr = xT_s.rearrange("p (c t) -> p c t", c=6)
        xT_dr = xT_d.rearrange("(c p) t -> p c t", p=128)
        nc.sync.dma_start(xT_sr[:, :, 0:512], xT_dr[:, :, 0:512])  # tb0
        nc.sync.dma_start(wall_s[:, 768:2432], wall_d[:, 768:2432])
        nc.sync.dma_start(xT_sr[:, :, 512:1024], xT_dr[:, :, 512:1024])
        ball_s = P.tile([128, 1], f32, name="ball")
        nc.sync.dma_start(ball_s[:, :], ball_d)
        qb_s = ball_s[:, 0:1]

        relpack_s = P.tile([128, 2 * HW], bf16, name="relpack")
        relh16_s = relpack_s[:, 0:HW]
        relw_s = relpack_s[:, HW:2 * HW]
        Kp = [P.tile([128, HW], bf16, name=f"Kp{j}") for j in range(2)]
        Qp = [P.tile([128, HW], bf16, name=f"Qp{j}") for j in range(2)]
        # j0 start-critical tables
        nc.sync.dma_start(relpack_s[0:64, HW:2 * HW],
                          relpack_d[0:64, HW:2 * HW])
        nc.sync.dma_start(relpack_s[0:64, 0:2048], relpack_d[0:64, 0:2048])
        nc.sync.dma_start(Kp[0][64:128, :], ohkw_d)
        ohkh8_s = P.tile([32, 2 * HW], f8e4, name="ohkh8")
        nc.sync.dma_start(ohkh8_s[:, :], ohkh8_d)
        # token-blocks 2..7 as three merged pair DMAs
        for tb2 in range(1, 4):
            nc.sync.dma_start(
                xT_sr[:, :, tb2 * 1024: tb2 * 1024 + 1024],
                xT_dr[:, :, tb2 * 1024: tb2 * 1024 + 1024])
        # j1 + half-1 tables (needed from ~50us on)
        nc.sync.dma_start(relpack_s[64:128, 0:2048],
                          relpack_d[64:128, 0:2048])
        nc.sync.dma_start(relpack_s[64:128, HW:2 * HW],
                          relpack_d[64:128, HW:2 * HW])
        nc.sync.dma_start(Kp[1][0:64, :], ohkw_d)
        nc.sync.dma_start(relpack_s[0:64, 2048:4096],
                          relpack_d[0:64, 2048:4096])
        projT2_s = P.tile([128, 768], bf16, name="projT2")
        nc.sync.dma_start(projT2_s[:, :], projT2_d)

        # ---- persistent computed tensors ----
        # RelHT8[j][p, i*RHW[j] + q] = 16*rel_h[q, kh=i*32+p] in fp8
        RelHT8 = [P.tile([32, 2 * RHW[j]], f8e4,
                         name=f"RelHT8{j}") for j in range(2)]
        Vp = [P.tile([128, 65 * 32], bf16, name=f"Vp{j}") for j in range(2)]
        # half-0 ctx staging: 16 qsubs x [j0 d 0:64 | j1 d 0:64]
        stg0 = P.tile([128, 16 * 128], bf16, name="stg0")
        # half-1 ctx staging: 16 qsubs x [j0 d 0:64]
        stg1 = P.tile([128, 16 * 64], bf16, name="stg1")
        # transposed, proj-ready rhs: (hd j0|j1, tokens)
        projrhs0 = P.tile([128, 2048], bf16, name="projrhs0")
        projrhs1 = P.tile([64, 2048], bf16, name="projrhs1")

        ones_s = P.tile([65, 64], bf16, name="ones")
        nc.vector.memset(ones_s[64:65, :], 1.0)
        warm_row = P.tile([65, 512], bf16, name="warmrow")
        nc.vector.memset(warm_row[64:65, :], 1.0)
        zz = P.tile([1, 512], bf16, name="zz")
        nc.vector.memset(zz[:, :], 0.0)
        for j in range(2):
            vp_r = Vp[j].rearrange("p (b e) -> p b e", e=65)
            nc.vector.memset(vp_r[:, :, 64:65], 1.0)

        with tc.tile_pool(name="ps", bufs=2, space="PSUM") as PS, \
             tc.tile_pool(name="pc", bufs=2, space="PSUM") as PC, \
             tc.tile_pool(name="paux", bufs=2, space="PSUM") as PAUX, \
             tc.tile_pool(name="esb", bufs=3) as ES, \
             tc.tile_pool(name="pos", bufs=8) as POS, \
             tc.tile_pool(name="post", bufs=2) as POST:

            # PE warm-up during the initial DMA wait
            wt = PAUX.tile([64, 512], f32, name="warmt", tag="aux")
            for _w in range(6):
                nc.tensor.matmul(wt[:, :], ones_s[64:65, :],
                                 warm_row[64:65, :], start=True, stop=True,
                                 tile_position=(64, 0))

            # ---- phase-1 work emitters (also injected mid-attention) ----
            def k_emit(t, pool=None):
                pool = pool or PAUX
                ts = slice(t * 512, t * 512 + 512)
                pk = pool.tile([128, 512], f32, name="pk",
                               tag="ctx" if pool is PC else "aux")
                for i in range(6):
                    nc.tensor.matmul(pk[:, :], wk_s[i][:, :], xT[i][:, ts],
                                     start=(i == 0), stop=(i == 5))
                if t < 2:
                    nc.scalar.activation(Kp[0][0:64, ts], pk[0:64, :],
                                         AF.Copy)
                    nc.scalar.activation(Kp[1][64:128, ts], pk[64:128, :],
                                         AF.Copy)
                else:
                    nc.vector.tensor_copy(Kp[0][0:64, ts], pk[0:64, :])
                    nc.vector.tensor_copy(Kp[1][64:128, ts], pk[64:128, :])

            def q_emit(t, pool=None):
                pool = pool or PAUX
                ts = slice(t * 512, t * 512 + 512)
                pq = pool.tile([128, 512], f32, name="pq",
                               tag="ctx" if pool is PC else "aux")
                for i in range(6):
                    nc.tensor.matmul(pq[:, :], wq_s[i][:, :], xT[i][:, ts],
                                     start=(i == 0), stop=(i == 5))
                nc.vector.tensor_scalar_add(Qp[0][0:64, ts], pq[0:64, :],
                                            qb_s[0:64, :])
                if t < 4:  # local head 1 only serves q-half 0
                    if t < 2:  # head: split across engines for latency
                        nc.scalar.add(Qp[1][64:128, ts], pq[64:128, :],
                                      qb_s[64:128, :])
                    else:
                        nc.vector.tensor_scalar_add(Qp[1][64:128, ts],
                                                    pq[64:128, :],
                                                    qb_s[64:128, :])

            def v_emit(b):
                bs = slice(b * 128, b * 128 + 128)
                pv = PAUX.tile([128, 128], f32, name="pv", tag="aux")
                for i in range(6):
                    nc.tensor.matmul(pv[:, :], xT[i][:, bs], wv_s[i][:, :],
                                     start=(i == 0), stop=(i == 5))
                for j in range(2):
                    nc.vector.tensor_copy(Vp[j][:, b * 65: b * 65 + 64],
                                          pv[:, j * 64: j * 64 + 64])

            def relw_emit(j, r, early=False):
                # RelW^T[kw, q] for token range r*1024..+1024 into the spare
                # rows of Q'
                rows = slice(0, 64) if j == 0 else slice(64, 128)
                orows = slice(64, 128) if j == 0 else slice(0, 64)
                qp_r = Qp[j].rearrange("p (qh qw) -> p qw qh", qw=64)
                qhs = slice(r * 16, r * 16 + 16)
                for h in range(2):
                    if early and h == 1:
                        pr = PC.tile([128, 512], f32, name="prw", tag="ctx")
                    else:
                        pr = PAUX.tile([128, 512], f32, name="prw", tag="aux")
                    for qi in range(32):
                        qw = h * 32 + qi
                        nc.tensor.matmul(
                            pr[orows, qi * 16: qi * 16 + 16],
                            relw_s[rows, qw * 64: qw * 64 + 64],
                            qp_r[rows, qw, qhs], start=True, stop=True)
                    src = pr[orows, :].rearrange("p (qw qh) -> p qw qh",
                                                 qh=16)
                    dst = qp_r[orows, h * 32:(h + 1) * 32, qhs]
                    if early and h == 0:
                        nc.scalar.activation(dst, src, AF.Copy)
                    else:
                        nc.vector.tensor_copy(dst, src)

            def relht8_emit(j, g, early=False):
                # RelHT8[j][p, i*RHWj + g*512..+512] = 16*rel_h[q, i*32+p]
                rows = slice(0, 64) if j == 0 else slice(64, 128)
                pra = PAUX.tile([32, 512], f32, name="pra", tag="aux")
                prb = (PC if early else PAUX).tile(
                    [32, 512], f32, name="prb",
                    tag="ctx" if early else "aux")
                for qi in range(8):
                    qh = g * 8 + qi
                    qs = slice(qh * 64, qh * 64 + 64)
                    for i, pr in enumerate((pra, prb)):
                        nc.tensor.matmul(
                            pr[:, qi * 64: qi * 64 + 64],
                            relh16_s[rows,
                                     qh * 64 + i * 32: qh * 64 + i * 32 + 32],
                            Qp[j][rows, qs], start=True, stop=True)
                hwj = RHW[j]
                for i, pr in enumerate((pra, prb)):
                    dst = RelHT8[j][:,
                                    i * hwj + g * 512: i * hwj + g * 512 + 512]
                    if early and i == 0:
                        nc.scalar.activation(dst, pr[:, :], AF.Copy)
                    else:
                        nc.vector.tensor_copy(dst, pr[:, :])

            def transp_grp(half, grp):
                # transpose 4 qsubs of ctx staging into proj-ready layout
                stg, prhs = (stg0, projrhs0) if half == 0 else (stg1, projrhs1)
                w = 128 if half == 0 else 64
                nprt = 128 if half == 0 else 64
                tp = PAUX.tile([nprt, 512], bf16, tag="aux", name="tp")
                for qi in range(4):
                    qq = grp * 4 + qi
                    nc.tensor.transpose(tp[:, qi * 128: qi * 128 + 128],
                                        stg[:, qq * w: qq * w + w], ident_s)
                nc.vector.tensor_copy(prhs[:, grp * 512: grp * 512 + 512],
                                      tp[:, :])

            pos_tiles = {}
            # DRAM view (p, ocb, t): one merged DMA covers all 6 ocb blocks
            outT_r = outT_d.rearrange("(o p) t -> p o t", p=128)

            def proj_sub(half, sub, ocb_range=range(6), tail=False):
                prhs = projrhs0 if half == 0 else projrhs1
                rows = slice(0, 128) if half == 0 else slice(0, 64)
                for n, ocb in enumerate(ocb_range):
                    # tail: ctx accumulators are draining, so their freed PC
                    # banks widen the PSUM rotation
                    if tail and ocb % 2 == 1:
                        po = PC.tile([128, 512], f32, tag="ctx", name="po")
                    else:
                        po = PAUX.tile([128, 512], f32, tag="aux", name="po")
                    nc.tensor.matmul(
                        po[:, :], projT2_s[rows, ocb * 128: ocb * 128 + 128],
                        prhs[:, sub * 512: sub * 512 + 512],
                        start=True, stop=True)
                    if tail:
                        # last-quarter path: one tile per sub, copies spread
                        # over all three idle engines, single merged DMA
                        if (half, sub, "t") not in pos_tiles:
                            pos_tiles[(half, sub, "t")] = POST.tile(
                                [128, 6 * 512], bf16, tag="post", name="post")
                        po_s = pos_tiles[(half, sub, "t")]
                        dst = po_s[:, ocb * 512: ocb * 512 + 512]
                        if n % 2 == 0:
                            nc.scalar.activation(dst, po[:, :], AF.Copy)
                        else:
                            nc.vector.tensor_copy(dst, po[:, :])
                        if ocb % 2 == 1:
                            ts = slice(half * 2048 + sub * 512,
                                       half * 2048 + sub * 512 + 512)
                            ob = slice(ocb - 1, ocb + 1)
                            nc.sync.dma_start(
                                outT_r[:, ob, ts],
                                po_s.rearrange("p (o t) -> p o t",
                                               o=6)[:, ob])
                        continue
                    if (half, ocb) not in pos_tiles or sub % 2 == 0:
                        pos_tiles[(half, ocb)] = POS.tile(
                            [128, 1024], bf16, tag="pos", name="pos")
                    po_s = pos_tiles[(half, ocb)]
                    nc.vector.tensor_copy(
                        po_s[:, (sub % 2) * 512: (sub % 2) * 512 + 512],
                        po[:, :])
                    if sub % 2 == 1:
                        nc.sync.dma_start(
                            outT_d[ocb * 128: ocb * 128 + 128,
                                   half * 2048 + (sub - 1) * 512:
                                   half * 2048 + (sub + 1) * 512],
                            po_s[:, :])

            def av_emit(ctx_ps, e_t, vp, pkb):
                # ONE accumulation group per ctx tile: interleaved start=True
                # groups in a shared PSUM bank wipe each other's first
                # contribution on real HW (each start zeroes the whole 2KB
                # zero region). The tiles are pre-zeroed by an opener matmul.
                for qs in range(8):
                    cp = ctx_ps[qs // 4]
                    co = (qs % 4) * 65
                    nc.tensor.matmul(
                        cp[:, co: co + 65],
                        e_t[:, qs * 128: qs * 128 + 128],
                        vp[:, pkb * 65: pkb * 65 + 65],
                        start=False, stop=(pkb == 31 and qs % 4 == 3),
                        skip_group_check=True)

            def attention_unit(uidx, j, half, inject=None, post_norm=None):
                # inject: {(qt, kb): [thunk, ...]} — extra PE-stream work
                # emitted mid-loop so it executes in exp-wait gaps.
                # post_norm: {qs: [thunk]} — tail pipelining hooks applied
                # after normalize of the given qt-1 qsub (normalize of qt 1
                # is also split across DVE and the now-idle ScalarE).
                inject = inject or {}
                rel8 = RelHT8[j].rearrange("p (i q) -> p i q", i=2)
                oh8 = ohkh8_s.rearrange("p (i k) -> p i k", i=2)
                for qt in range(2):
                    qtb = half * 2048 + qt * 1024   # global q base
                    qrb = qtb if j == 0 else qt * 1024  # rel-table base
                    ctx_ps = [PC.tile([128, 260], f32, tag="ctx",
                                      name=f"ctxps{_s}") for _s in range(2)]
                    for cp in ctx_ps:  # zeroing opener: start the bank once
                        nc.tensor.matmul(cp[:, 0:260], zz[0:1, 0:128],
                                         zz[0:1, 128:388], start=True,
                                         stop=False, skip_group_check=True)
                    pend = []
                    for kb in range(32):
                        kbs = slice(kb * 128, kb * 128 + 128)
                        ps_t = PS.tile([128, 1024], f32, tag="s")
                        for s in range(2):
                            qs = slice(qtb + s * 512, qtb + s * 512 + 512)
                            nc.tensor.matmul(
                                ps_t[:, s * 512: s * 512 + 512],
                                Kp[j][:, kbs], Qp[j][:, qs],
                                start=True, stop=False)
                        for s in range(2):
                            qr = slice(qrb + s * 512, qrb + s * 512 + 512)
                            if USE_FP8_REL:
                                nc.tensor.matmul(
                                    ps_t[:, s * 512: s * 512 + 512],
                                    oh8[:, :, kbs], rel8[:, :, qr],
                                    start=False, stop=True,
                                    perf_mode=PM.DoubleRow)
                            else:
                                nc.tensor.matmul(
                                    ps_t[:, s * 512: s * 512 + 512],
                                    oh8[:, 0, kbs], rel8[:, 0, qr],
                                    start=False, stop=False)
                                nc.tensor.matmul(
                                    ps_t[:, s * 512: s * 512 + 512],
                                    oh8[:, 1, kbs], rel8[:, 1, qr],
                                    start=False, stop=True)
                        e_t = ES.tile([128, 1024], bf16, tag="e", bufs=9)
                        nc.scalar.activation(e_t[:, :], ps_t[:, :], AF.Exp)
                        pend.append((kb, e_t))
                        for fn in inject.get((qt, kb), ()):
                            fn()
                        if len(pend) > 5:
                            pkb, pe_t = pend.pop(0)
                            av_emit(ctx_ps, pe_t, Vp[j], pkb)
                    for pkb, pe_t in pend:
                        av_emit(ctx_ps, pe_t, Vp[j], pkb)
                    # normalize q-major ctx into staging; one strided
                    # reciprocal per ctx tile covers its 4 denominators
                    rcp = ES.tile([128, 8], f32, tag="r", bufs=4)
                    for i, cp in enumerate(ctx_ps):
                        den = cp.rearrange("p (q e) -> p q e", e=65)[:, :, 64]
                        nc.vector.reciprocal(rcp[:, i * 4: i * 4 + 4], den)
                    for qs in range(8):
                        cp = ctx_ps[qs // 4]
                        co = (qs % 4) * 65
                        qq = qt * 8 + qs
                        if half == 0:
                            dst = stg0[:, qq * 128 + j * 64:
                                       qq * 128 + j * 64 + 64]
                        else:
                            dst = stg1[:, qq * 64: qq * 64 + 64]
                        if post_norm and qt == 1 and qs % 2 == 1:
                            nc.scalar.mul(dst, cp[:, co: co + 64],
                                          rcp[:, qs: qs + 1])
                        else:
                            nc.vector.tensor_scalar_mul(
                                dst, cp[:, co: co + 64], rcp[:, qs: qs + 1])
                        if post_norm and qt == 1:
                            for fn in post_norm.get(qs, ()):
                                fn()

            # ---- phase 1 proper: minimum work before u0 can stream ----
            k_emit(0)
            q_emit(0)
            q_emit(1)
            relw_emit(0, 0, early=True)
            relht8_emit(0, 0, early=True)
            relht8_emit(0, 1, early=True)

            def _i(fn, *a, **kw):
                return lambda: fn(*a, **kw)

            # u0 = (j0, h0): fill exp-wait gaps with the rest of phase 1.
            inj0 = {}
            # k1 feeds S(kb4..7); injected so it doesn't gate the first exp
            inj0.setdefault((0, 0), []).append(_i(k_emit, 1))
            for b in range(12):
                # V(b) must precede its first AV consumer (same qt, kb=b+6)
                inj0.setdefault((0, b // 2), []).append(_i(v_emit, b))
            for b in range(12, 32):
                inj0.setdefault((0, b - 6), []).append(_i(v_emit, b))
            # K(t) feeds qt0's own S stream; Q(t2,t3) feeds qt1's tables.
            # Q(t4..7) is only needed by u1-injected tables + u2, so it
            # moves to qt1 where the PE has slack under the exp stream.
            ktq = [_i(k_emit, 2), _i(q_emit, 2), _i(k_emit, 3),
                   _i(q_emit, 3), _i(k_emit, 4), _i(k_emit, 5),
                   _i(k_emit, 6), _i(k_emit, 7)]
            for n, fn in enumerate(ktq):
                inj0.setdefault((0, 2 * n + 3), []).append(fn)
            for n in range(4):
                inj0.setdefault((1, 8 * n + 3), []).append(_i(q_emit, 4 + n))
            # tables for qt1 (q 1024:2048), finishing a few kbs early
            inj0.setdefault((0, 26), []).append(_i(relw_emit, 0, 1))
            inj0.setdefault((0, 28), []).append(_i(relht8_emit, 0, 2))
            inj0.setdefault((0, 30), []).append(_i(relht8_emit, 0, 3))
            # j1 tables for u1; spread thin across qt1
            for n, fn in enumerate([_i(relw_emit, 1, 0),
                                    _i(relht8_emit, 1, 0),
                                    _i(relht8_emit, 1, 1),
                                    _i(relw_emit, 1, 1),
                                    _i(relht8_emit, 1, 2),
                                    _i(relht8_emit, 1, 3)]):
                inj0.setdefault((1, 5 * n + 1), []).append(fn)
            attention_unit(0, *sched[0], inject=inj0)

            # u1 = (j1, h0): qt0 takes j0's half-1 tables (for u2); qt1
            # takes the first half of the half-0 transposes + projection
            # (u0's and u1-qt0's staging is complete by then)
            inj1 = {}
            for n, fn in enumerate([_i(relht8_emit, 0, 4),
                                    _i(relht8_emit, 0, 5),
                                    _i(relht8_emit, 0, 6),
                                    _i(relht8_emit, 0, 7),
                                    _i(relw_emit, 0, 2),
                                    _i(relw_emit, 0, 3)]):
                inj1.setdefault((0, 5 * n + 2), []).append(fn)
            inj1[(1, 1)] = [_i(transp_grp, 0, 0)]
            inj1[(1, 3)] = [_i(transp_grp, 0, 1)]
            for sub in range(2):
                for oc3 in range(3):
                    inj1.setdefault((1, 6 + sub * 6 + oc3 * 2), []).append(
                        _i(proj_sub, 0, sub, range(oc3 * 2, oc3 * 2 + 2)))
            attention_unit(1, *sched[1], inject=inj1)

            # u2 = (j0, h1): qt0 finishes half-0 proj; qt1 does half-1's
            # first half; the last quarter rides the post-norm tail hooks.
            inj2 = {}
            inj2[(0, 1)] = [_i(transp_grp, 0, 2)]
            inj2[(0, 3)] = [_i(transp_grp, 0, 3)]
            for sub in (2, 3):
                for oc3 in range(3):
                    inj2.setdefault((0, 6 + (sub - 2) * 6 + oc3 * 2),
                                    []).append(
                        _i(proj_sub, 0, sub, range(oc3 * 2, oc3 * 2 + 2)))
            inj2[(1, 1)] = [_i(transp_grp, 1, 0)]
            inj2[(1, 3)] = [_i(transp_grp, 1, 1)]
            for sub in range(2):
                for oc3 in range(3):
                    inj2.setdefault((1, 6 + sub * 6 + oc3 * 2), []).append(
                        _i(proj_sub, 1, sub, range(oc3 * 2, oc3 * 2 + 2)))
            post = {
                3: [_i(transp_grp, 1, 2)],
                4: [_i(proj_sub, 1, 2, range(0, 3), True)],
                5: [_i(proj_sub, 1, 2, range(3, 6), True)],
                7: [_i(transp_grp, 1, 3),
                    _i(proj_sub, 1, 3, range(0, 6), True)],
            }
            attention_unit(2, *sched[2], inject=inj2, post_norm=post)

    nc.compile()
    return nc


def kernel(x, qkv_w, qkv_b, proj_w, proj_b, rel_pos_h, rel_pos_w, num_heads):
    global LAST_EXEC_NS, _PROGRAM
    from concourse.bass_utils import run_bass_kernel_spmd

    x = np.asarray(x, dtype=np.float32)
    qkv_w = np.asarray(qkv_w, dtype=np.float32)
    qkv_b = np.asarray(qkv_b, dtype=np.float32)
    proj_w = np.asarray(proj_w, dtype=np.float32)
    proj_b = np.asarray(proj_b, dtype=np.float32)
    rel_pos_h = np.asarray(rel_pos_h, dtype=np.float32)
    rel_pos_w = np.asarray(rel_pos_w, dtype=np.float32)
    assert int(num_heads) == NH

    in_maps = [_prep_core_inputs(c, x, qkv_w, qkv_b, proj_w,
                                 rel_pos_h, rel_pos_w) for c in range(NCORES)]

    if _PROGRAM is None:
        _PROGRAM = _build_program()
    nc = _PROGRAM

    import os
    trace = os.environ.get("KERNEL_TRACE", "0") == "1"
    try:
        res = run_bass_kernel_spmd(nc, in_maps, core_ids=list(range(NCORES)),
                                   trace=trace)
    except ModuleNotFoundError:
        res = run_bass_kernel_spmd(nc, in_maps, core_ids=list(range(NCORES)),
                                   trace=False)
    LAST_EXEC_NS = res.exec_time_ns

    out = np.zeros((DIM, HW), dtype=np.float32)
    for c in range(NCORES):
        o = np.asarray(res.results[c]["outT"], dtype=np.float32)
        if c % 2 == 1:  # un-swap token halves
            o = np.concatenate([o[:, 2048:], o[:, :2048]], axis=1)
        out += o
    # v-bias is token-independent after projection: fold into the bias
    proj_b_eff = proj_b + proj_w @ qkv_b[1536:2304]
    out = out.T + proj_b_eff[None, :]
    return out.reshape(1, Hh, Ww, DIM).astype(np.float32)
